# revision 7
# baseline (speedup 1.0000x reference)
"""Trainium2 Bass kernel: transformer block with sliding-window GQA attention
and a dense top-2-of-8 MoE feed-forward, data-parallel over 8 NeuronCores.

Sharding: each core owns half of one batch sequence (512 query tokens), plus
256 history tokens so the 256-wide sliding-window attention needs no
cross-core communication.  All large matmuls run in bf16 with fp32
accumulation; the MoE gate path stays fp32 so expert routing matches the
fp32 reference.  Outputs are gathered on the host into the full [4,1024,1024]
tensor.
"""

import os
import numpy as np
import ml_dtypes

# ---------------- problem constants (hardcoded from the reference model) ----
B, T, C = 4, 1024, 1024
NH, NKV, HD = 16, 4, 64
E, TOPK, FF = 8, 2, 4096
WIN = 256
EPS = 1e-6

NCORES = 8
TOK = 512            # query tokens per core
HIST = 256           # history rows ahead of the queries
BUF = TOK + HIST     # key/value rows per core
KW = 384             # key window per 128-query tile
P = 128

BF16 = ml_dtypes.bfloat16

# Head-slot permutation: q head in slot s must sit at the same 64-partition
# offset as its kv head (g = head//4) so the scores matmul sees matching base
# partitions.  Even slots hold heads with even g, odd slots heads with odd g.
SLOT_TO_HEAD = []
_A = [0, 1, 2, 3, 8, 9, 10, 11]   # g in {0,2}
_B = [4, 5, 6, 7, 12, 13, 14, 15]  # g in {1,3}
for _i in range(8):
    SLOT_TO_HEAD.append(_A[_i])
    SLOT_TO_HEAD.append(_B[_i])
G_OF_SLOT = [SLOT_TO_HEAD[s] // 4 for s in range(16)]

_prog_cache = {}
LAST_EXEC_NS = None
LAST_RESULTS = None


def _build_program():
    import concourse.bass as bass
    import concourse.bacc as bacc
    import concourse.tile as tile
    from concourse import mybir
    from concourse.masks import make_identity
    from contextlib import ExitStack

    f32 = mybir.dt.float32
    bf16 = mybir.dt.bfloat16
    ALU = mybir.AluOpType
    ACTF = mybir.ActivationFunctionType
    AX = mybir.AxisListType

    nc = bacc.Bacc(None, target_bir_lowering=False, debug=False)

    # ---------------- DRAM parameters (per-core inputs) ----------------
    d_xhist = nc.declare_dram_parameter("xhist", [HIST, C], f32, isOutput=False)
    d_xq = nc.declare_dram_parameter("xq", [TOK, C], f32, isOutput=False)
    d_wq = nc.declare_dram_parameter("wq", [C, NH * HD], bf16, isOutput=False)
    d_wk = nc.declare_dram_parameter("wk", [C, NKV * HD], bf16, isOutput=False)
    d_wv = nc.declare_dram_parameter("wv", [C, NKV * HD], bf16, isOutput=False)
    d_wo = nc.declare_dram_parameter("wo", [C, C], bf16, isOutput=False)
    d_gw = nc.declare_dram_parameter("gate_w", [C, E], f32, isOutput=False)
    d_w1 = nc.declare_dram_parameter("w1", [E, C, FF], bf16, isOutput=False)
    d_w3 = nc.declare_dram_parameter("w3", [E, C, FF], bf16, isOutput=False)
    d_w2 = nc.declare_dram_parameter("w2", [E, FF, C], bf16, isOutput=False)
    d_cosq = nc.declare_dram_parameter("cosq", [TOK, C], bf16, isOutput=False)
    d_sinq = nc.declare_dram_parameter("sinq", [TOK, C], bf16, isOutput=False)
    d_cosk = nc.declare_dram_parameter("cosk", [BUF, NKV * HD], bf16, isOutput=False)
    d_sink = nc.declare_dram_parameter("sink", [BUF, NKV * HD], bf16, isOutput=False)
    d_mask = nc.declare_dram_parameter("mask", [4, P, KW], f32, isOutput=False)
    d_out = nc.declare_dram_parameter("out", [TOK, C], f32, isOutput=True)

    NQT = TOK // P            # 4 query-row tiles
    NBT = BUF // P            # 6 buffer-row tiles
    NCT = C // P              # 8 channel tiles

    with ExitStack() as ctx:
        tc = ctx.enter_context(tile.TileContext(nc))
        ps = ctx.enter_context(tc.tile_pool(name="ps", bufs=4, space="PSUM"))
        ps_tr = ctx.enter_context(tc.tile_pool(name="ps_tr", bufs=2, space="PSUM"))
        const = ctx.enter_context(tc.tile_pool(name="const", bufs=1))
        glob = ctx.enter_context(tc.tile_pool(name="glob", bufs=1))

        ident_bf = const.tile([P, P], bf16, tag="ident_bf")
        make_identity(nc, ident_bf)
        ident_f32 = const.tile([P, P], f32, tag="ident_f32")
        make_identity(nc, ident_f32)
        eps_ap = const.tile([P, 1], f32, tag="eps")
        nc.vector.memset(eps_ap[:, :], EPS)

        # persistent across the whole kernel
        h_sb = glob.tile([P, NQT, C], f32, tag="h")        # residual stream / final acc
        gT_bf = glob.tile([P, NCT, TOK], bf16, tag="gTbf")  # g transposed, bf16
        comb = glob.tile([P, NQT, E], f32, tag="comb")      # per-token expert weights

        def rmsnorm_scale(wpl, xin, tag):
            """Returns an AP [P,1] with 1/sqrt(mean(x^2)+eps) for a [P,C] input."""
            stats = wpl.tile([P, 2, 6], f32, tag="bnstats")
            xr = xin.rearrange("p (s d) -> p s d", s=2)
            for s in range(2):
                nc.vector.bn_stats(out=stats[:, s, :], in_=xr[:, s, :])
            mv = wpl.tile([P, 2], f32, tag="bnmv")
            nc.vector.bn_aggr(out=mv[:, :], in_=stats[:, :, :])
            # mean(x^2) = var + mean^2
            msq = wpl.tile([P, 1], f32, tag=tag + "_msq")
            nc.vector.scalar_tensor_tensor(
                out=msq[:, :], in0=mv[:, 0:1], scalar=mv[:, 0:1], in1=mv[:, 1:2],
                op0=ALU.mult, op1=ALU.add)
            std = wpl.tile([P, 1], f32, tag=tag + "_std")
            nc.scalar.activation(out=std[:, :], in_=msq[:, :], func=ACTF.Sqrt,
                                 bias=eps_ap[:, :], scale=1.0)
            rs = wpl.tile([P, 1], f32, tag=tag + "_rs")
            nc.vector.reciprocal(out=rs[:, :], in_=std[:, :])
            return rs

        # ============ scope 1: everything up to the MoE =====================
        with ExitStack() as s_cd:
            cd = s_cd.enter_context(tc.tile_pool(name="cd", bufs=1))
            qT = cd.tile([P, NCT, TOK], bf16, tag="qT")      # [16h x 64d, 512]
            kT = cd.tile([P, NKV // 2, BUF], bf16, tag="kT")  # [4kv x 64d, 768]
            v_sb = cd.tile([P, NBT, NKV * HD], bf16, tag="v")
            xq_sb = cd.tile([P, NQT, C], f32, tag="xq")
            nc.sync.dma_start(out=xq_sb[:, :, :],
                              in_=d_xq[:, :].rearrange("(n p) c -> p n c", p=P))

            with ExitStack() as s_ab:
                ab = s_ab.enter_context(tc.tile_pool(name="ab", bufs=1))
                work = s_ab.enter_context(tc.tile_pool(name="workab", bufs=2))
                hnT = ab.tile([P, NCT, BUF], bf16, tag="hnT")
                wq_sb = ab.tile([P, NCT, NH * HD], bf16, tag="wq")
                wk_sb = ab.tile([P, NCT, NKV * HD], bf16, tag="wk")
                wv_sb = ab.tile([P, NCT, NKV * HD], bf16, tag="wv")
                xh_sb = ab.tile([P, HIST // P, C], f32, tag="xhist")
                nc.sync.dma_start(out=wq_sb[:, :, :],
                                  in_=d_wq[:, :].rearrange("(n p) m -> p n m", p=P))
                nc.sync.dma_start(out=wk_sb[:, :, :],
                                  in_=d_wk[:, :].rearrange("(n p) m -> p n m", p=P))
                nc.sync.dma_start(out=wv_sb[:, :, :],
                                  in_=d_wv[:, :].rearrange("(n p) m -> p n m", p=P))
                nc.sync.dma_start(out=xh_sb[:, :, :],
                                  in_=d_xhist[:, :].rearrange("(n p) c -> p n c", p=P))

                # ---- phase A: attention rmsnorm + transpose to hnT [C, BUF]
                for it in range(NBT):
                    xin = xh_sb[:, it, :] if it < 2 else xq_sb[:, it - 2, :]
                    rs = rmsnorm_scale(work, xin, "n1")
                    hn = work.tile([P, C], bf16, tag="hn")
                    nc.vector.tensor_scalar(out=hn[:, :], in0=xin, scalar1=rs[:, :],
                                            scalar2=None, op0=ALU.mult)
                    for c in range(NCT):
                        pt = ps_tr.tile([P, P], bf16, tag="tr")
                        nc.tensor.transpose(pt[:, :], hn[:, c * P:(c + 1) * P], ident_bf[:, :])
                        nc.vector.tensor_copy(out=hnT[:, c, it * P:(it + 1) * P], in_=pt[:, :])

                # ---- phase B: q/k/v projections + RoPE + transposes
                # q: per query-row tile, two 512-wide halves (8 heads each)
                for mt in range(NQT):
                    cosm = work.tile([P, C], bf16, tag="cosm")
                    sinm = work.tile([P, C], bf16, tag="sinm")
                    nc.sync.dma_start(out=cosm[:, :], in_=d_cosq[mt * P:(mt + 1) * P, :])
                    nc.sync.dma_start(out=sinm[:, :], in_=d_sinq[mt * P:(mt + 1) * P, :])
                    for hlf in range(2):
                        pq = ps.tile([P, 512], f32, tag="ps")
                        for k in range(NCT):
                            nc.tensor.matmul(
                                pq[:, :],
                                hnT[:, k, HIST + mt * P: HIST + (mt + 1) * P],
                                wq_sb[:, k, hlf * 512:(hlf + 1) * 512],
                                start=(k == 0), stop=(k == NCT - 1))
                        # RoPE on 8 heads: within each 64-wide head, rotate halves
                        pqh = pq[:, :].rearrange("p (h d) -> p h d", h=8)
                        rr = work.tile([P, 512], f32, tag="rr")
                        rrh = rr[:, :].rearrange("p (h d) -> p h d", h=8)
                        nc.vector.tensor_scalar(out=rrh[:, :, 0:32], in0=pqh[:, :, 32:64],
                                                scalar1=-1.0, scalar2=None, op0=ALU.mult)
                        nc.vector.tensor_copy(out=rrh[:, :, 32:64], in_=pqh[:, :, 0:32])
                        nc.vector.tensor_mul(rr[:, :], rr[:, :],
                                             sinm[:, hlf * 512:(hlf + 1) * 512])
                        qf = work.tile([P, 512], f32, tag="qf")
                        nc.vector.tensor_mul(qf[:, :], pq[:, :],
                                             cosm[:, hlf * 512:(hlf + 1) * 512])
                        qro = work.tile([P, 512], bf16, tag="qro")
                        nc.vector.tensor_add(qro[:, :], qf[:, :], rr[:, :])
                        for c in range(4):
                            pt = ps_tr.tile([P, P], bf16, tag="tr")
                            nc.tensor.transpose(pt[:, :], qro[:, c * P:(c + 1) * P],
                                                ident_bf[:, :])
                            nc.vector.tensor_copy(
                                out=qT[:, hlf * 4 + c, mt * P:(mt + 1) * P], in_=pt[:, :])

                # k and v: all 6 buffer-row tiles
                for mt in range(NBT):
                    coskm = work.tile([P, NKV * HD], bf16, tag="coskm")
                    sinkm = work.tile([P, NKV * HD], bf16, tag="sinkm")
                    nc.sync.dma_start(out=coskm[:, :], in_=d_cosk[mt * P:(mt + 1) * P, :])
                    nc.sync.dma_start(out=sinkm[:, :], in_=d_sink[mt * P:(mt + 1) * P, :])
                    pk = ps.tile([P, NKV * HD], f32, tag="ps")
                    for k in range(NCT):
                        nc.tensor.matmul(pk[:, :], hnT[:, k, mt * P:(mt + 1) * P],
                                         wk_sb[:, k, :],
                                         start=(k == 0), stop=(k == NCT - 1))
                    pkh = pk[:, :].rearrange("p (h d) -> p h d", h=NKV)
                    rr = work.tile([P, NKV * HD], f32, tag="rrk")
                    rrh = rr[:, :].rearrange("p (h d) -> p h d", h=NKV)
                    nc.vector.tensor_scalar(out=rrh[:, :, 0:32], in0=pkh[:, :, 32:64],
                                            scalar1=-1.0, scalar2=None, op0=ALU.mult)
                    nc.vector.tensor_copy(out=rrh[:, :, 32:64], in_=pkh[:, :, 0:32])
                    nc.vector.tensor_mul(rr[:, :], rr[:, :], sinkm[:, :])
                    kf = work.tile([P, NKV * HD], f32, tag="kf")
                    nc.vector.tensor_mul(kf[:, :], pk[:, :], coskm[:, :])
                    kro = work.tile([P, NKV * HD], bf16, tag="kro")
                    nc.vector.tensor_add(kro[:, :], kf[:, :], rr[:, :])
                    for c in range(2):
                        pt = ps_tr.tile([P, P], bf16, tag="tr")
                        nc.tensor.transpose(pt[:, :], kro[:, c * P:(c + 1) * P],
                                            ident_bf[:, :])
                        nc.vector.tensor_copy(out=kT[:, c, mt * P:(mt + 1) * P],
                                              in_=pt[:, :])
                    # v projection (no rope), natural layout
                    pv = ps.tile([P, NKV * HD], f32, tag="ps")
                    for k in range(NCT):
                        nc.tensor.matmul(pv[:, :], hnT[:, k, mt * P:(mt + 1) * P],
                                         wv_sb[:, k, :],
                                         start=(k == 0), stop=(k == NCT - 1))
                    nc.vector.tensor_copy(out=v_sb[:, mt, :], in_=pv[:, :])
            # ---- s_ab closed: projection weights / hnT freed

            with ExitStack() as s_c:
                cpool = s_c.enter_context(tc.tile_pool(name="cpool", bufs=1))
                work = s_c.enter_context(tc.tile_pool(name="workc", bufs=2))
                yT = cpool.tile([P, NCT, TOK], bf16, tag="yT")
                wo_sb = cpool.tile([P, NCT, C], bf16, tag="wo")
                mask_sb = cpool.tile([P, NQT, KW], f32, tag="mask")
                nc.sync.dma_start(out=wo_sb[:, :, :],
                                  in_=d_wo[:, :].rearrange("(n p) m -> p n m", p=P))
                nc.sync.dma_start(out=mask_sb[:, :, :],
                                  in_=d_mask[:, :, :].rearrange("q p k -> p q k"))

                # ---- phase C: attention per (head-slot, query tile)
                for s in range(16):
                    g = G_OF_SLOT[s]
                    off = (s % 2) * 64
                    for qi in range(NQT):
                        Qs = qi * P
                        psc = ps.tile([P, KW], f32, tag="ps")
                        nc.tensor.matmul(
                            psc[:, :],
                            qT[off:off + 64, s // 2, Qs:Qs + P],
                            kT[off:off + 64, g // 2, Qs:Qs + KW],
                            start=True, stop=True)
                        sm = work.tile([P, KW], f32, tag="sm")
                        nc.vector.scalar_tensor_tensor(
                            out=sm[:, :], in0=psc[:, :], scalar=0.125,
                            in1=mask_sb[:, qi, :], op0=ALU.mult, op1=ALU.add)
                        mx = work.tile([P, 1], f32, tag="mx")
                        nc.vector.tensor_reduce(out=mx[:, :], in_=sm[:, :],
                                                axis=AX.X, op=ALU.max)
                        negmx = work.tile([P, 1], f32, tag="negmx")
                        nc.vector.tensor_scalar(out=negmx[:, :], in0=mx[:, :],
                                                scalar1=-1.0, scalar2=None, op0=ALU.mult)
                        pat = work.tile([P, KW], bf16, tag="pat")
                        rsum = work.tile([P, 1], f32, tag="rsum")
                        nc.scalar.activation(out=pat[:, :], in_=sm[:, :], func=ACTF.Exp,
                                             bias=negmx[:, :], scale=1.0,
                                             accum_out=rsum[:, :])
                        rinv = work.tile([P, 1], f32, tag="rinv")
                        nc.vector.reciprocal(out=rinv[:, :], in_=rsum[:, :])
                        nc.vector.tensor_scalar(out=pat[:, :], in0=pat[:, :],
                                                scalar1=rinv[:, :], scalar2=None,
                                                op0=ALU.mult)
                        att = work.tile([P, 3, P], bf16, tag="att")
                        for j in range(3):
                            pt = ps_tr.tile([P, P], bf16, tag="tr")
                            nc.tensor.transpose(pt[:, :], pat[:, j * P:(j + 1) * P],
                                                ident_bf[:, :])
                            nc.vector.tensor_copy(out=att[:, j, :], in_=pt[:, :])
                        py = ps.tile([P, P], f32, tag="ps")
                        for j in range(3):
                            nc.tensor.matmul(
                                py[off:off + 64, :],
                                v_sb[:, qi + j, g * HD:(g + 1) * HD],
                                att[:, j, :],
                                start=(j == 0), stop=(j == 2))
                        nc.scalar.copy(out=yT[off:off + 64, s // 2, Qs:Qs + P],
                                       in_=py[off:off + 64, :])

                # ---- phase D: wo projection + residual -> h
                for mt in range(NQT):
                    for hlf in range(2):
                        po = ps.tile([P, 512], f32, tag="ps")
                        for k in range(NCT):
                            nc.tensor.matmul(
                                po[:, :], yT[:, k, mt * P:(mt + 1) * P],
                                wo_sb[:, k, hlf * 512:(hlf + 1) * 512],
                                start=(k == 0), stop=(k == NCT - 1))
                        nc.vector.tensor_add(h_sb[:, mt, hlf * 512:(hlf + 1) * 512],
                                             po[:, :],
                                             xq_sb[:, mt, hlf * 512:(hlf + 1) * 512])
            # s_c closed

        # ============ phase E: ffn rmsnorm, g transposes, gate + comb ========
        with ExitStack() as s_e:
            ep = s_e.enter_context(tc.tile_pool(name="ep", bufs=1))
            work = s_e.enter_context(tc.tile_pool(name="worke", bufs=2))
            gT32 = ep.tile([P, NCT, TOK], f32, tag="gT32")
            gw_sb = ep.tile([P, NCT, E], f32, tag="gw")
            nc.sync.dma_start(out=gw_sb[:, :, :],
                              in_=d_gw[:, :].rearrange("(n p) e -> p n e", p=P))
            for mt in range(NQT):
                rs = rmsnorm_scale(work, h_sb[:, mt, :], "n2")
                g32 = work.tile([P, C], f32, tag="g32")
                nc.vector.tensor_scalar(out=g32[:, :], in0=h_sb[:, mt, :],
                                        scalar1=rs[:, :], scalar2=None, op0=ALU.mult)
                for c in range(NCT):
                    pt = ps_tr.tile([P, P], f32, tag="tr")
                    nc.tensor.transpose(pt[:, :], g32[:, c * P:(c + 1) * P],
                                        ident_f32[:, :])
                    nc.vector.tensor_copy(out=gT32[:, c, mt * P:(mt + 1) * P],
                                          in_=pt[:, :])
                    nc.vector.tensor_copy(out=gT_bf[:, c, mt * P:(mt + 1) * P],
                                          in_=pt[:, :])

            # gate logits (fp32) + top-2 combine weights
            for mt in range(NQT):
                pg = ps.tile([P, E], f32, tag="ps")
                for k in range(NCT):
                    nc.tensor.matmul(pg[:, :], gT32[:, k, mt * P:(mt + 1) * P],
                                     gw_sb[:, k, :],
                                     start=(k == 0), stop=(k == NCT - 1))
                m1 = work.tile([P, 1], f32, tag="m1")
                nc.vector.tensor_reduce(out=m1[:, :], in_=pg[:, :], axis=AX.X, op=ALU.max)
                negm1 = work.tile([P, 1], f32, tag="negm1")
                nc.vector.tensor_scalar(out=negm1[:, :], in0=m1[:, :], scalar1=-1.0,
                                        scalar2=None, op0=ALU.mult)
                ex = work.tile([P, E], f32, tag="ex")
                nc.scalar.activation(out=ex[:, :], in_=pg[:, :], func=ACTF.Exp,
                                     bias=negm1[:, :], scale=1.0)
                is1 = work.tile([P, E], f32, tag="is1")
                nc.vector.tensor_scalar(out=is1[:, :], in0=ex[:, :], scalar1=1.0,
                                        scalar2=None, op0=ALU.is_ge)
                exm = work.tile([P, E], f32, tag="exm")
                nc.vector.tensor_sub(exm[:, :], ex[:, :], is1[:, :])
                m2 = work.tile([P, 1], f32, tag="m2")
                nc.vector.tensor_reduce(out=m2[:, :], in_=exm[:, :], axis=AX.X, op=ALU.max)
                sel = work.tile([P, E], f32, tag="sel")
                nc.vector.tensor_scalar(out=sel[:, :], in0=ex[:, :], scalar1=m2[:, :],
                                        scalar2=None, op0=ALU.is_ge)
                exsel = work.tile([P, E], f32, tag="exsel")
                nc.vector.tensor_mul(exsel[:, :], ex[:, :], sel[:, :])
                ssum = work.tile([P, 1], f32, tag="ssum")
                nc.vector.tensor_reduce(out=ssum[:, :], in_=exsel[:, :], axis=AX.X,
                                        op=ALU.add)
                sinv = work.tile([P, 1], f32, tag="sinv")
                nc.vector.reciprocal(out=sinv[:, :], in_=ssum[:, :])
                nc.vector.tensor_scalar(out=comb[:, mt, :], in0=exsel[:, :],
                                        scalar1=sinv[:, :], scalar2=None, op0=ALU.mult)
        # s_e closed

        # ============ phase F: MoE experts ==================================
        NT1 = 8          # 512-wide ff blocks for w1/w3
        with ExitStack() as s_m:
            mp = s_m.enter_context(tc.tile_pool(name="mp", bufs=1))
            wpool = s_m.enter_context(tc.tile_pool(name="wpool", bufs=2))
            w2pool = s_m.enter_context(tc.tile_pool(name="w2pool", bufs=1))
            work = s_m.enter_context(tc.tile_pool(name="workm", bufs=2))
            hidT = mp.tile([P, FF // P, TOK], bf16, tag="hidT")
            tmp_acc = mp.tile([P, NQT, C], f32, tag="tmpacc")
            for e in range(E):
                # --- hid = silu(g@w1) * (g@w3), built transposed into hidT
                for nt in range(NT1):
                    w1b = wpool.tile([P, NCT, 512], bf16, tag="w1b")
                    w3b = wpool.tile([P, NCT, 512], bf16, tag="w3b")
                    nc.sync.dma_start(
                        out=w1b[:, :, :],
                        in_=d_w1[e, :, nt * 512:(nt + 1) * 512]
                        .rearrange("(n p) f -> p n f", p=P))
                    nc.sync.dma_start(
                        out=w3b[:, :, :],
                        in_=d_w3[e, :, nt * 512:(nt + 1) * 512]
                        .rearrange("(n p) f -> p n f", p=P))
                    hblk = wpool.tile([P, NQT, 512], bf16, tag="hblk")
                    for mt in range(NQT):
                        psA = ps.tile([P, 512], f32, tag="ps")
                        psB = ps.tile([P, 512], f32, tag="ps")
                        for k in range(NCT):
                            lhs = gT_bf[:, k, mt * P:(mt + 1) * P]
                            nc.tensor.matmul(psA[:, :], lhs, w1b[:, k, :],
                                             start=(k == 0), stop=(k == NCT - 1))
                            nc.tensor.matmul(psB[:, :], lhs, w3b[:, k, :],
                                             start=(k == 0), stop=(k == NCT - 1))
                        s1 = work.tile([P, 512], bf16, tag="s1")
                        nc.scalar.activation(out=s1[:, :], in_=psA[:, :], func=ACTF.Silu)
                        nc.vector.tensor_mul(hblk[:, mt, :], s1[:, :], psB[:, :])
                    for c in range(4):
                        pt = ps_tr.tile([P, 512], bf16, tag="tr")
                        for mt in range(NQT):
                            nc.tensor.transpose(pt[:, mt * P:(mt + 1) * P],
                                                hblk[:, mt, c * P:(c + 1) * P],
                                                ident_bf[:, :])
                        nc.vector.tensor_copy(out=hidT[:, nt * 4 + c, :], in_=pt[:, :])

                # --- out += comb[:,e] * (hid @ w2[e]), streamed in two halves
                for kh in range(2):
                    w2h = w2pool.tile([P, 16, C], bf16, tag="w2h")
                    nc.sync.dma_start(
                        out=w2h[:, :, :],
                        in_=d_w2[e, kh * 2048:(kh + 1) * 2048, :]
                        .rearrange("(n p) c -> p n c", p=P))
                    for mt in range(NQT):
                        for hlf in range(2):
                            psC = ps.tile([P, 512], f32, tag="ps")
                            for kf in range(16):
                                nc.tensor.matmul(
                                    psC[:, :],
                                    hidT[:, kh * 16 + kf, mt * P:(mt + 1) * P],
                                    w2h[:, kf, hlf * 512:(hlf + 1) * 512],
                                    start=(kf == 0), stop=(kf == 15))
                            osl = h_sb[:, mt, hlf * 512:(hlf + 1) * 512]
                            tsl = tmp_acc[:, mt, hlf * 512:(hlf + 1) * 512]
                            if kh == 0:
                                nc.scalar.copy(out=tsl, in_=psC[:, :])
                            else:
                                tt = work.tile([P, 512], f32, tag="tt")
                                nc.vector.tensor_add(tt[:, :], psC[:, :], tsl)
                                nc.vector.scalar_tensor_tensor(
                                    out=osl, in0=tt[:, :],
                                    scalar=comb[:, mt, e:e + 1], in1=osl,
                                    op0=ALU.mult, op1=ALU.add)

        # final store
        nc.sync.dma_start(out=d_out[:, :].rearrange("(n p) c -> p n c", p=P),
                          in_=h_sb[:, :, :])

    nc.compile()
    return nc


def _host_prepare(inputs):
    """Builds the 8 per-core input maps from the full-problem inputs."""
    x = np.asarray(inputs["x"], np.float32)
    attn_w = np.asarray(inputs["attn_norm_w"], np.float32)
    ffn_w = np.asarray(inputs["ffn_norm_w"], np.float32)
    wq = np.asarray(inputs["wq"], np.float32) * attn_w[:, None]
    wk = np.asarray(inputs["wk"], np.float32) * attn_w[:, None]
    wv = np.asarray(inputs["wv"], np.float32) * attn_w[:, None]
    wo = np.asarray(inputs["wo"], np.float32)
    gate_w = np.asarray(inputs["gate_w"], np.float32) * ffn_w[:, None]
    w1 = np.asarray(inputs["w1"], np.float32) * ffn_w[None, :, None]
    w3 = np.asarray(inputs["w3"], np.float32) * ffn_w[None, :, None]
    w2 = np.asarray(inputs["w2"], np.float32)

    # permute q heads into slots, and wo rows to match
    wq_p = np.empty_like(wq)
    wo_p = np.empty_like(wo)
    for s, h in enumerate(SLOT_TO_HEAD):
        wq_p[:, s * HD:(s + 1) * HD] = wq[:, h * HD:(h + 1) * HD]
        wo_p[s * HD:(s + 1) * HD, :] = wo[h * HD:(h + 1) * HD, :]

    wq_b = wq_p.astype(BF16)
    wk_b = wk.astype(BF16)
    wv_b = wv.astype(BF16)
    wo_b = wo_p.astype(BF16)
    w1_b = w1.astype(BF16)
    w3_b = w3.astype(BF16)
    w2_b = w2.astype(BF16)

    inv_freq = 1.0 / (10000.0 ** (np.arange(0, HD, 2, dtype=np.float32) / HD))

    def cos_sin(positions, nheads):
        freqs = positions[:, None].astype(np.float32) * inv_freq[None, :]
        emb = np.concatenate([freqs, freqs], axis=-1)       # [n, HD]
        c = np.tile(np.cos(emb), (1, nheads)).astype(BF16)
        s = np.tile(np.sin(emb), (1, nheads)).astype(BF16)
        return c, s

    in_maps = []
    for core in range(NCORES):
        b, hf = core // 2, core % 2
        start = hf * TOK
        xq = x[b, start:start + TOK]
        if hf == 0:
            xhist = np.zeros((HIST, C), np.float32)
        else:
            xhist = x[b, start - HIST:start]

        qpos = np.arange(start, start + TOK)
        kpos = np.arange(start - HIST, start + TOK)
        cosq, sinq = cos_sin(qpos, NH)
        cosk, sink = cos_sin(kpos, NKV)

        # additive mask [4, 128, KW]: key buffer row r = Qs + j,
        # allowed iff i < j <= i + WIN and (row real: Qs + j >= HIST for hf=0)
        mask = np.full((4, P, KW), -1e9, np.float32)
        ii = np.arange(P)[:, None]
        jj = np.arange(KW)[None, :]
        for qi in range(4):
            ok = (jj > ii) & (jj <= ii + WIN)
            if hf == 0:
                ok &= (qi * P + jj) >= HIST
            mask[qi][ok] = 0.0

        in_maps.append({
            "xhist": np.ascontiguousarray(xhist),
            "xq": np.ascontiguousarray(xq),
            "wq": wq_b, "wk": wk_b, "wv": wv_b, "wo": wo_b,
            "gate_w": gate_w, "w1": w1_b, "w3": w3_b, "w2": w2_b,
            "cosq": cosq, "sinq": sinq, "cosk": cosk, "sink": sink,
            "mask": mask,
        })
    return in_maps


def _install_ntff_shim():
    """Makes antenv.axon_hooks importable and registers the NTFF profile
    hook so run_bass_kernel_spmd(trace=True) works in this container."""
    import sys as _sys
    import types as _types
    if "antenv.axon_hooks" in _sys.modules:
        return
    try:
        import antenv
        mod = _types.ModuleType("antenv.axon_hooks")
        mod._hook = None
        mod.set_axon_ntff_profile_hook = lambda h: setattr(mod, "_hook", h)
        mod.get_axon_ntff_profile_hook = lambda: mod._hook
        _sys.modules["antenv.axon_hooks"] = mod
        antenv.axon_hooks = mod
        from trn_agent_boot.trn_boot import _ntff_profile_via_ctypes
        hook = _ntff_profile_via_ctypes("/opt/axon/libaxon_pjrt.so")
        if hook is not None:
            mod._hook = hook
    except Exception:
        pass


def kernel(**inputs):
    global LAST_EXEC_NS, LAST_RESULTS
    from concourse.bass_utils import run_bass_kernel_spmd
    _install_ntff_shim()

    if "nc" not in _prog_cache:
        _prog_cache["nc"] = _build_program()
    nc = _prog_cache["nc"]

    in_maps = _host_prepare(inputs)
    res = run_bass_kernel_spmd(
        nc, in_maps, list(range(NCORES)),
        trace=bool(os.environ.get("BASS_TRACE")),
    )
    LAST_RESULTS = res
    LAST_EXEC_NS = res.exec_time_ns

    out = np.empty((B, T, C), np.float32)
    for core in range(NCORES):
        b, hf = core // 2, core % 2
        out[b, hf * TOK:(hf + 1) * TOK] = res.results[core]["out"]
    return out
